# revision 1
# baseline (speedup 1.0000x reference)
"""Cumulative linear multihead attention (KV prefix-scan) on 8 TRN2 NeuronCores.

Sharding: 4 sequence(tb)-groups x 2 head-groups. Core c = hg*4 + g handles
t-range [g*256,(g+1)*256) for both batches and heads [hg*8, hg*8+8).
Per core: column-parallel in_proj for its heads over its tb rows, chunked
linear attention (chunk=128) with the cross-core KV prefix state exchanged
via an 8-core AllGather, then a row/column partial out_proj. Host sums the
two head-group partials per tb row.
"""
import numpy as np
import ml_dtypes

import concourse.bass as bass
import concourse.mybir as mybir
import concourse.tile as tile
from concourse.tile import ScopedClock
from concourse.bass_utils import run_bass_kernel_spmd

T, B, E, H, D = 1024, 2, 1024, 16, 64
TB = T * B
N_CORES = 8
TBG = 4        # tb groups
HGS = 2        # head groups
TBC = TB // TBG          # 512 tb rows per core
DHC = (H // HGS) * D     # 512 head dims per core per projection
NP = (H // HGS) * B      # 16 (b,h) pairs per core
C = 128                  # chunk
NCH = TBC // (B * C)     # 2 chunks per (b,h) per core
BF = mybir.dt.bfloat16
F32 = mybir.dt.float32


_MAXW = 1  # this walrus build allows a single sync-wait condition per instruction


def _split_excess_waits(nc):
    """Hoist sync waits beyond _MAXW onto same-engine NOPs placed just before
    the over-constrained instruction (engine streams execute in list order)."""
    n_spliced = 0
    for fn in nc.m.functions:
        for bb in fn.blocks:
            insts = bb.instructions
            i = 0
            while i < len(insts):
                ins = insts[i]
                si = getattr(ins, "sync_info", None)
                if si is not None and len(si.on_wait) > _MAXW:
                    waits = list(si.on_wait)
                    keep = waits[-_MAXW:]
                    extra = waits[:-_MAXW]
                    for j in range(0, len(extra), _MAXW):
                        nop = mybir.InstNoOp(
                            name=f"waitsplit_{n_spliced}",
                            engine=ins.engine,
                            bass_nofuse=True,
                            sync_info=mybir.SyncInfo(
                                on_wait=extra[j : j + _MAXW], on_update=[]
                            ),
                        )
                        insts.insert(i, nop)
                        i += 1
                        n_spliced += 1
                    ins.sync_info = mybir.SyncInfo(
                        on_wait=keep, on_update=list(si.on_update)
                    )
                i += 1
    return n_spliced


_NC_CACHE = {}


def _build_nc():
    if "nc" in _NC_CACHE:
        return _NC_CACHE["nc"]
    nc = bass.Bass()
    xtq = nc.dram_tensor("xtq", [E, TBC], BF, kind="ExternalInput")
    xtk = nc.dram_tensor("xtk", [E, TBC], BF, kind="ExternalInput")
    xtv = nc.dram_tensor("xtv", [E, TBC], BF, kind="ExternalInput")
    wqT = nc.dram_tensor("wqT", [E, DHC], BF, kind="ExternalInput")
    wkT = nc.dram_tensor("wkT", [E, DHC], BF, kind="ExternalInput")
    wvT = nc.dram_tensor("wvT", [E, DHC], BF, kind="ExternalInput")
    woT = nc.dram_tensor("woT", [DHC, E], BF, kind="ExternalInput")
    maskd = nc.dram_tensor("maskd", [C, C], F32, kind="ExternalInput")
    coefsd = nc.dram_tensor("coefsd", [128, N_CORES], F32, kind="ExternalInput")
    pout = nc.dram_tensor("pout", [TBC, E], F32, kind="ExternalOutput")
    cc_in = nc.dram_tensor("cc_in", [D, NP * D], BF)
    cc_shared = nc.dram_tensor(
        "cc_shared", [N_CORES * D, NP * D], BF, addr_space="Shared"
    )

    mult = mybir.AluOpType.mult
    from concourse.tile import add_dep_helper

    with tile.TileContext(nc) as tc:
        with (
            tc.tile_pool(name="wpool", bufs=1) as wpool,
            tc.tile_pool(name="actpool", bufs=1) as actpool,
            tc.tile_pool(name="stpool", bufs=1) as stpool,
            tc.tile_pool(name="ampool", bufs=1) as ampool,
            tc.tile_pool(name="obuf", bufs=3) as obuf,
            tc.tile_pool(name="ps_big", bufs=2, space="PSUM") as ps_big,
            tc.tile_pool(name="ps_kv", bufs=2, space="PSUM") as ps_kv,
            tc.tile_pool(name="ps_at", bufs=2, space="PSUM") as ps_at,
            tc.tile_pool(name="ps_io", bufs=2, space="PSUM") as ps_io,
        ):
            def load_tiles(src, n, w, nm):
                ts = []
                for k in range(n):
                    t = wpool.tile([128, w], BF, name=f"{nm}{k}")
                    nc.sync.dma_start(out=t[:], in_=src[k * 128 : (k + 1) * 128, :])
                    ts.append(t)
                return ts

            # k/v-side loads first: they gate the L states -> exchange
            xk_sb = load_tiles(xtk, 8, TBC, "xk")
            xv_sb = load_tiles(xtv, 8, TBC, "xv")
            wk_sb = load_tiles(wkT, 8, DHC, "wk")
            wv_sb = load_tiles(wvT, 8, DHC, "wv")

            def proj_rows(x_tiles, w_tiles, nm):
                outs = []
                for i in range(4):
                    ps = ps_big.tile([128, DHC], F32, name="ps_proj")
                    for k in range(8):
                        nc.tensor.matmul(
                            ps[:],
                            lhsT=x_tiles[k][:, i * 128 : (i + 1) * 128],
                            rhs=w_tiles[k][:],
                            start=(k == 0),
                            stop=(k == 7),
                        )
                    o = actpool.tile([128, DHC], BF, name=f"{nm}{i}")
                    nc.vector.tensor_copy(out=o[:], in_=ps[:])
                    outs.append(o)
                return outs

            def proj_cols(x_tiles, w_tiles, nm):
                outs = []
                for j in range(4):
                    ps = ps_big.tile([128, TBC], F32, name="ps_proj")
                    for k in range(8):
                        nc.tensor.matmul(
                            ps[:],
                            lhsT=w_tiles[k][:, j * 128 : (j + 1) * 128],
                            rhs=x_tiles[k][:],
                            start=(k == 0),
                            stop=(k == 7),
                        )
                    o = actpool.tile([128, TBC], BF, name=f"{nm}{j}")
                    nc.vector.tensor_copy(out=o[:], in_=ps[:])
                    outs.append(o)
                return outs

            kS_sb = proj_rows(xk_sb, wk_sb, "kS")
            v_sb = proj_rows(xv_sb, wv_sb, "v")

            # ---- local KV chunk states ----
            kv0_all = stpool.tile([D, NP * D], F32, name="kv0_all")
            kv1_all = stpool.tile([D, NP * D], F32, name="kv1_all")
            for b in range(B):
                for c in range(NCH):
                    it = b * 2 + c
                    ps = ps_kv.tile([D, 8 * D], F32, name="ps_kv")
                    for h in range(8):
                        nc.tensor.matmul(
                            ps[:, h * D : (h + 1) * D],
                            lhsT=kS_sb[it][:, h * D : (h + 1) * D],
                            rhs=v_sb[it][:, h * D : (h + 1) * D],
                            start=True,
                            stop=True,
                        )
                    dst = kv0_all if c == 0 else kv1_all
                    nc.vector.tensor_copy(
                        out=dst[:, b * 8 * D : (b + 1) * 8 * D], in_=ps[:]
                    )
            l_bf = stpool.tile([D, NP * D], BF, name="l_bf")
            nc.vector.tensor_add(out=l_bf[:], in0=kv0_all[:], in1=kv1_all[:])

            # ---- exchange: bf16 L-state allgather ----
            nc.sync.dma_start(out=cc_in[:], in_=l_bf[:])
            nc.gpsimd.collective_compute(
                "AllGather",
                mybir.AluOpType.bypass,
                replica_groups=[list(range(N_CORES))],
                ins=[cc_in[:]],
                outs=[cc_shared[:]],
            )

            # remaining inputs (loads overlap L/exchange)
            xq_sb = load_tiles(xtq, 8, TBC, "xq")
            wq_sb = load_tiles(wqT, 8, DHC, "wq")
            wo_sb = load_tiles(woT, 4, E, "wo")
            mask_sb = wpool.tile([C, C], F32, name="mask_sb")
            nc.sync.dma_start(out=mask_sb[:], in_=maskd[:])
            coefs_sb = wpool.tile([128, N_CORES], F32, name="coefs_sb")
            nc.sync.dma_start(out=coefs_sb[:], in_=coefsd[:])

            qT_sb = proj_cols(xq_sb, wq_sb, "qT")
            kT_sb = proj_cols(xk_sb, wk_sb, "kT")

            # ---- A^T + mask ----
            am_sb = {}
            for p in range(NP):
                b, h = divmod(p, NP // B)
                jj, ro = divmod(h, 2)
                ro *= D
                for c in range(NCH):
                    col = b * 256 + c * 128
                    ps = ps_at.tile([C, C], F32, name="ps_at")
                    nc.tensor.matmul(
                        ps[:],
                        lhsT=kT_sb[jj][ro : ro + D, col : col + C],
                        rhs=qT_sb[jj][ro : ro + D, col : col + C],
                        start=True,
                        stop=True,
                    )
                    am = ampool.tile([C, C], BF, name=f"am{p}_{c}")
                    nc.vector.tensor_tensor(
                        out=am[:], in0=ps[:], in1=mask_sb[:], op=mult
                    )
                    am_sb[(p, c)] = am

            # ---- read slots (after barrier), cast to f32 via gpsimd DMA ----
            cc_sb = []
            for i in range(N_CORES):
                t = stpool.tile([D, NP * D], F32, name=f"cc{i}")
                nc.gpsimd.dma_start(
                    out=t[:], in_=cc_shared[i * D : (i + 1) * D, :]
                )
                cc_sb.append(t)
            pcur = stpool.tile([D, NP * D], F32, name="pfx0")
            nc.vector.memset(pcur[:], 0.0)
            for cid in range(N_CORES):
                pnxt = stpool.tile([D, NP * D], F32, name=f"pfx{cid+1}")
                nc.vector.scalar_tensor_tensor(
                    out=pnxt[:],
                    in0=cc_sb[cid][:],
                    scalar=coefs_sb[0:D, cid : cid + 1],
                    in1=pcur[:],
                    op0=mult,
                    op1=mybir.AluOpType.add,
                )
                pcur = pnxt
            s1f = stpool.tile([D, NP * D], F32, name="s1f")
            nc.vector.tensor_add(out=s1f[:], in0=pcur[:], in1=kv0_all[:])
            s0b = stpool.tile([128, NP * D], BF, name="s0b")
            s1b = stpool.tile([128, NP * D], BF, name="s1b")
            nc.vector.tensor_copy(out=s0b[0:D, :], in_=pcur[:])
            nc.vector.tensor_copy(out=s0b[D : 2 * D, :], in_=pcur[:])
            nc.vector.tensor_copy(out=s1b[0:D, :], in_=s1f[:])
            nc.vector.tensor_copy(out=s1b[D : 2 * D, :], in_=s1f[:])

            # ---- intra + inter -> outT ----
            outT_sb = {
                (j, i): actpool.tile([128, 128], BF, name=f"outT{j}_{i}")
                for j in range(4)
                for i in range(4)
            }
            for p in range(NP):
                b, h = divmod(p, NP // B)
                jj, ro = divmod(h, 2)
                ro *= D
                for c in range(NCH):
                    it = b * 2 + c
                    col = b * 256 + c * 128
                    ps = ps_io.tile([D, C], F32, name="ps_io")
                    nc.tensor.matmul(
                        ps[:],
                        lhsT=v_sb[it][:, h * D : (h + 1) * D],
                        rhs=am_sb[(p, c)][:],
                        start=True,
                        stop=False,
                    )
                    sb = s0b if c == 0 else s1b
                    nc.tensor.matmul(
                        ps[:],
                        lhsT=sb[ro : ro + D, p * D : (p + 1) * D],
                        rhs=qT_sb[jj][ro : ro + D, col : col + C],
                        start=False,
                        stop=True,
                    )
                    nc.vector.tensor_copy(
                        out=outT_sb[(jj, col // 128)][ro : ro + D, :], in_=ps[:]
                    )

            # ---- out_proj partial ----
            for i in range(4):
                for n in range(2):
                    ps = ps_big.tile([128, 512], F32, name="ps_proj")
                    for k in range(4):
                        nc.tensor.matmul(
                            ps[:],
                            lhsT=outT_sb[(k, i)][:, :],
                            rhs=wo_sb[k][:, n * 512 : (n + 1) * 512],
                            start=(k == 0),
                            stop=(k == 3),
                        )
                    ob = obuf.tile([128, 512], F32, name="ob")
                    nc.vector.tensor_copy(out=ob[:], in_=ps[:])
                    nc.sync.dma_start(
                        out=pout[i * 128 : (i + 1) * 128, n * 512 : (n + 1) * 512],
                        in_=ob[:],
                    )
    _split_excess_waits(nc)
    _NC_CACHE["nc"] = nc
    return nc


def _bf16(x):
    return np.ascontiguousarray(x, dtype=ml_dtypes.bfloat16)


def kernel(
    query,
    key_,
    value,
    in_proj_weight,
    in_proj_bias,
    out_proj_bias,
    out_proj_weight=None,
    **kw,
):
    # tolerate arbitrary kw order; pull required arrays
    if out_proj_weight is None:
        out_proj_weight = kw["out_proj_weight"]
    query = np.asarray(query, np.float32)
    key_ = np.asarray(key_, np.float32)
    value = np.asarray(value, np.float32)
    W = np.asarray(in_proj_weight, np.float32)
    Wo = np.asarray(out_proj_weight, np.float32)
    bi = np.asarray(in_proj_bias, np.float32)
    bo = np.asarray(out_proj_bias, np.float32)
    assert not np.any(bi), "nonzero in_proj_bias unsupported by this kernel"

    scale = np.float32(1.0 / np.sqrt(D))
    wq, wk, wv = W[:E], W[E : 2 * E], W[2 * E :]

    # (E, BT) b-major transposed activations
    XTq = np.ascontiguousarray(query.transpose(2, 1, 0).reshape(E, TB))
    XTk = np.ascontiguousarray(key_.transpose(2, 1, 0).reshape(E, TB))
    XTv = np.ascontiguousarray(value.transpose(2, 1, 0).reshape(E, TB))

    mask = np.triu(np.ones((C, C), np.float32))  # U[s,t]=1 iff s<=t

    in_maps = []
    for core in range(N_CORES):
        hg, g = divmod(core, TBG)
        cols = np.r_[g * 256 : (g + 1) * 256, T + g * 256 : T + (g + 1) * 256]
        hsl = slice(hg * DHC, (hg + 1) * DHC)
        coefs = np.zeros((128, N_CORES), np.float32)
        for cid in range(N_CORES):
            if cid // TBG == hg and cid % TBG < g:
                coefs[:, cid] = 1.0
        in_maps.append(
            {
                "xtq": _bf16(XTq[:, cols]),
                "xtk": _bf16(XTk[:, cols]),
                "xtv": _bf16(XTv[:, cols]),
                "wqT": _bf16((wq[hsl, :] * scale).T),
                "wkT": _bf16(wk[hsl, :].T),
                "wvT": _bf16(wv[hsl, :].T),
                "woT": _bf16(Wo[:, hsl].T.copy()),
                "maskd": mask,
                "coefsd": coefs,
            }
        )

    nc = _build_nc()
    res = run_bass_kernel_spmd(nc, in_maps, list(range(N_CORES)))

    out = np.zeros((T, B, E), np.float32)
    for core in range(N_CORES):
        hg, g = divmod(core, TBG)
        po = res.results[core]["pout"]  # (512, 1024) rows: b*256 + tl
        for b in range(B):
            out[g * 256 : (g + 1) * 256, b, :] += po[b * 256 : (b + 1) * 256, :]
    out += bo
    return out



# revision 18
# speedup vs baseline: 46.2618x; 46.2618x over previous
"""Cumulative linear multihead attention (KV prefix-scan) on 8 TRN2 NeuronCores.

Sharding: 4 sequence(tb)-groups x 2 head-groups. Core c = hg*4 + g handles
t-range [g*256,(g+1)*256) for both batches and heads [hg*8, hg*8+8).
Per core: column-parallel in_proj for its heads over its tb rows, chunked
linear attention (chunk=128) with the cross-core KV prefix state exchanged
via an 8-core AllGather, then a row/column partial out_proj. Host sums the
two head-group partials per tb row.
"""
import numpy as np
import ml_dtypes

import concourse.bass as bass
import concourse.mybir as mybir
import concourse.tile as tile
from concourse.bass_utils import run_bass_kernel_spmd

T, B, E, H, D = 1024, 2, 1024, 16, 64
TB = T * B
N_CORES = 8
TBG = 4        # tb groups
HGS = 2        # head groups
TBC = TB // TBG          # 512 tb rows per core
DHC = (H // HGS) * D     # 512 head dims per core per projection
NP = (H // HGS) * B      # 16 (b,h) pairs per core
C = 128                  # chunk
NCH = TBC // (B * C)     # 2 chunks per (b,h) per core
BF = mybir.dt.bfloat16
F32 = mybir.dt.float32


_MAXW = 1  # this walrus build allows a single sync-wait condition per instruction


def _split_excess_waits(nc):
    """Hoist sync waits beyond _MAXW onto same-engine NOPs placed just before
    the over-constrained instruction (engine streams execute in list order)."""
    n_spliced = 0
    for fn in nc.m.functions:
        for bb in fn.blocks:
            insts = bb.instructions
            i = 0
            while i < len(insts):
                ins = insts[i]
                si = getattr(ins, "sync_info", None)
                if si is not None and len(si.on_wait) > _MAXW:
                    waits = list(si.on_wait)
                    keep = waits[-_MAXW:]
                    extra = waits[:-_MAXW]
                    for j in range(0, len(extra), _MAXW):
                        nop = mybir.InstNoOp(
                            name=f"waitsplit_{n_spliced}",
                            engine=ins.engine,
                            bass_nofuse=True,
                            sync_info=mybir.SyncInfo(
                                on_wait=extra[j : j + _MAXW], on_update=[]
                            ),
                        )
                        insts.insert(i, nop)
                        i += 1
                        n_spliced += 1
                    ins.sync_info = mybir.SyncInfo(
                        on_wait=keep, on_update=list(si.on_update)
                    )
                i += 1
    return n_spliced


_NC_CACHE = {}


def _build_nc(reps: int = 1):
    if reps in _NC_CACHE:
        return _NC_CACHE[reps]
    nc = bass.Bass()
    xtq = nc.dram_tensor("xtq", [E, TBC], BF, kind="ExternalInput")
    xtk = nc.dram_tensor("xtk", [E, TBC], BF, kind="ExternalInput")
    xtv = nc.dram_tensor("xtv", [E, TBC], BF, kind="ExternalInput")
    wqT = nc.dram_tensor("wqT", [E, DHC], BF, kind="ExternalInput")
    wkT = nc.dram_tensor("wkT", [E, DHC], BF, kind="ExternalInput")
    wvT = nc.dram_tensor("wvT", [E, DHC], BF, kind="ExternalInput")
    woT = nc.dram_tensor("woT", [DHC, E], BF, kind="ExternalInput")
    maskd = nc.dram_tensor("maskd", [C, C], F32, kind="ExternalInput")
    coefsd = nc.dram_tensor("coefsd", [128, N_CORES], F32, kind="ExternalInput")
    pout = nc.dram_tensor("pout", [TBC, E], F32, kind="ExternalOutput")
    cc_in = nc.dram_tensor("cc_in", [D, NP * D], BF)
    cc_shared = nc.dram_tensor(
        "cc_shared", [N_CORES * D, NP * D], BF, addr_space="Shared"
    )

    mult = mybir.AluOpType.mult

    with tile.TileContext(nc) as tc:
        with (
            tc.tile_pool(name="wpool", bufs=(2 if reps > 1 else 1)) as wpool,
            tc.tile_pool(name="actpool", bufs=1) as actpool,
            tc.tile_pool(name="stpool", bufs=1) as stpool,
            tc.tile_pool(name="ampool", bufs=1) as ampool,
            tc.tile_pool(name="obuf", bufs=3) as obuf,
            tc.tile_pool(name="ps_big", bufs=2, space="PSUM") as ps_big,
            tc.tile_pool(name="ps_kv", bufs=2, space="PSUM") as ps_kv,
            tc.tile_pool(name="ps_at", bufs=2, space="PSUM") as ps_at,
            tc.tile_pool(name="ps_io", bufs=2, space="PSUM") as ps_io,
        ):
          for _rep in range(reps):
            def load_tiles(src, n, w, nm):
                ts = []
                for k in range(n):
                    t = wpool.tile([128, w], BF, name=f"{nm}{k}")
                    nc.sync.dma_start(out=t[:], in_=src[k * 128 : (k + 1) * 128, :])
                    ts.append(t)
                return ts

            # k/v-side loads first: they gate the L states -> exchange
            xk_sb = load_tiles(xtk, 8, TBC, "xk")
            xv_sb = load_tiles(xtv, 8, TBC, "xv")
            wk_sb = load_tiles(wkT, 8, DHC, "wk")
            wv_sb = load_tiles(wvT, 8, DHC, "wv")

            def proj_rows(x_tiles, w_tiles, nm):
                outs = []
                for i in range(4):
                    ps = ps_big.tile([128, DHC], F32, name="ps_proj")
                    for k in range(8):
                        nc.tensor.matmul(
                            ps[:],
                            lhsT=x_tiles[k][:, i * 128 : (i + 1) * 128],
                            rhs=w_tiles[k][:],
                            start=(k == 0),
                            stop=(k == 7),
                        )
                    o = actpool.tile([128, DHC], BF, name=f"{nm}{i}")
                    nc.vector.tensor_copy(out=o[:], in_=ps[:])
                    outs.append(o)
                return outs

            def proj_cols(x_tiles, w_tiles, nm):
                outs = []
                for j in range(4):
                    ps = ps_big.tile([128, TBC], F32, name="ps_proj")
                    for k in range(8):
                        nc.tensor.matmul(
                            ps[:],
                            lhsT=w_tiles[k][:, j * 128 : (j + 1) * 128],
                            rhs=x_tiles[k][:],
                            start=(k == 0),
                            stop=(k == 7),
                        )
                    o = actpool.tile([128, TBC], BF, name=f"{nm}{j}")
                    nc.vector.tensor_copy(out=o[:], in_=ps[:])
                    outs.append(o)
                return outs

            kS_sb = proj_rows(xk_sb, wk_sb, "kS")
            v_sb = proj_rows(xv_sb, wv_sb, "v")

            # ---- local KV chunk states ----
            kv0_all = stpool.tile([D, NP * D], F32, name="kv0_all")
            kv1_all = stpool.tile([D, NP * D], F32, name="kv1_all")
            for b in range(B):
                for c in range(NCH):
                    it = b * 2 + c
                    ps = ps_kv.tile([D, 8 * D], F32, name="ps_kv")
                    for h in range(8):
                        nc.tensor.matmul(
                            ps[:, h * D : (h + 1) * D],
                            lhsT=kS_sb[it][:, h * D : (h + 1) * D],
                            rhs=v_sb[it][:, h * D : (h + 1) * D],
                            start=True,
                            stop=True,
                        )
                    dst = kv0_all if c == 0 else kv1_all
                    nc.vector.tensor_copy(
                        out=dst[:, b * 8 * D : (b + 1) * 8 * D], in_=ps[:]
                    )
            l_bf = stpool.tile([D, NP * D], BF, name="l_bf")
            nc.vector.tensor_add(out=l_bf[:], in0=kv0_all[:], in1=kv1_all[:])

            # ---- exchange: bf16 L-state allgather ----
            nc.sync.dma_start(out=cc_in[:], in_=l_bf[:])
            nc.gpsimd.collective_compute(
                "AllGather",
                mybir.AluOpType.bypass,
                replica_groups=[list(range(N_CORES))],
                ins=[cc_in[:]],
                outs=[cc_shared[:]],
            )

            # remaining inputs (loads overlap L/exchange)
            xq_sb = load_tiles(xtq, 8, TBC, "xq")
            wq_sb = load_tiles(wqT, 8, DHC, "wq")
            wo_sb = load_tiles(woT, 4, E, "wo")
            mask_sb = wpool.tile([C, C], F32, name="mask_sb")
            nc.sync.dma_start(out=mask_sb[:], in_=maskd[:])
            coefs_sb = wpool.tile([128, N_CORES], F32, name="coefs_sb")
            nc.sync.dma_start(out=coefs_sb[:], in_=coefsd[:])

            qT_sb = proj_cols(xq_sb, wq_sb, "qT")
            kT_sb = proj_cols(xk_sb, wk_sb, "kT")

            # ---- A^T + mask ----
            am_sb = {}
            for p in range(NP):
                b, h = divmod(p, NP // B)
                jj, ro = divmod(h, 2)
                ro *= D
                for c in range(NCH):
                    col = b * 256 + c * 128
                    ps = ps_at.tile([C, C], F32, name="ps_at")
                    nc.tensor.matmul(
                        ps[:],
                        lhsT=kT_sb[jj][ro : ro + D, col : col + C],
                        rhs=qT_sb[jj][ro : ro + D, col : col + C],
                        start=True,
                        stop=True,
                    )
                    am = ampool.tile([C, C], BF, name=f"am{p}_{c}")
                    nc.vector.tensor_tensor(
                        out=am[:], in0=ps[:], in1=mask_sb[:], op=mult
                    )
                    am_sb[(p, c)] = am

            # ---- read slots (after barrier), cast to f32 via gpsimd DMA ----
            cc_sb = []
            for i in range(N_CORES):
                t = stpool.tile([D, NP * D], F32, name=f"cc{i}")
                nc.gpsimd.dma_start(
                    out=t[:], in_=cc_shared[i * D : (i + 1) * D, :]
                )
                cc_sb.append(t)
            pcur = stpool.tile([D, NP * D], F32, name="pfx0")
            nc.vector.memset(pcur[:], 0.0)
            for cid in range(N_CORES):
                pnxt = stpool.tile([D, NP * D], F32, name=f"pfx{cid+1}")
                nc.vector.scalar_tensor_tensor(
                    out=pnxt[:],
                    in0=cc_sb[cid][:],
                    scalar=coefs_sb[0:D, cid : cid + 1],
                    in1=pcur[:],
                    op0=mult,
                    op1=mybir.AluOpType.add,
                )
                pcur = pnxt
            s1f = stpool.tile([D, NP * D], F32, name="s1f")
            nc.vector.tensor_add(out=s1f[:], in0=pcur[:], in1=kv0_all[:])
            s0b = stpool.tile([128, NP * D], BF, name="s0b")
            s1b = stpool.tile([128, NP * D], BF, name="s1b")
            nc.vector.tensor_copy(out=s0b[0:D, :], in_=pcur[:])
            nc.vector.tensor_copy(out=s0b[D : 2 * D, :], in_=pcur[:])
            nc.vector.tensor_copy(out=s1b[0:D, :], in_=s1f[:])
            nc.vector.tensor_copy(out=s1b[D : 2 * D, :], in_=s1f[:])

            # ---- intra + inter -> outT ----
            outT_sb = {
                (j, i): actpool.tile([128, 128], BF, name=f"outT{j}_{i}")
                for j in range(4)
                for i in range(4)
            }
            for p in range(NP):
                b, h = divmod(p, NP // B)
                jj, ro = divmod(h, 2)
                ro *= D
                for c in range(NCH):
                    it = b * 2 + c
                    col = b * 256 + c * 128
                    ps = ps_io.tile([D, C], F32, name="ps_io")
                    nc.tensor.matmul(
                        ps[:],
                        lhsT=v_sb[it][:, h * D : (h + 1) * D],
                        rhs=am_sb[(p, c)][:],
                        start=True,
                        stop=False,
                    )
                    sb = s0b if c == 0 else s1b
                    nc.tensor.matmul(
                        ps[:],
                        lhsT=sb[ro : ro + D, p * D : (p + 1) * D],
                        rhs=qT_sb[jj][ro : ro + D, col : col + C],
                        start=False,
                        stop=True,
                    )
                    nc.vector.tensor_copy(
                        out=outT_sb[(jj, col // 128)][ro : ro + D, :], in_=ps[:]
                    )

            # ---- out_proj partial ----
            for i in range(4):
                for n in range(2):
                    ps = ps_big.tile([128, 512], F32, name="ps_proj")
                    for k in range(4):
                        nc.tensor.matmul(
                            ps[:],
                            lhsT=outT_sb[(k, i)][:, :],
                            rhs=wo_sb[k][:, n * 512 : (n + 1) * 512],
                            start=(k == 0),
                            stop=(k == 3),
                        )
                    ob = obuf.tile([128, 512], F32, name="ob")
                    nc.vector.tensor_copy(out=ob[:], in_=ps[:])
                    nc.sync.dma_start(
                        out=pout[i * 128 : (i + 1) * 128, n * 512 : (n + 1) * 512],
                        in_=ob[:],
                    )
    _split_excess_waits(nc)
    _NC_CACHE[reps] = nc
    return nc


class _Runner:
    """Compile a Bass module once via bass2jax/PJRT and allow repeated
    executions with device-resident inputs (no donation, no per-call
    retrace). Used by test.py for HW timing."""

    def __init__(self, nc):
        import jax
        from jax.sharding import Mesh, PartitionSpec, NamedSharding
        from jax.experimental.shard_map import shard_map
        from concourse.bass2jax import (
            _bass_exec_p,
            partition_id_tensor,
            install_neuronx_cc_hook,
        )

        install_neuronx_cc_hook()
        self.jax = jax
        partition_name = (
            nc.partition_id_tensor.name if nc.partition_id_tensor else None
        )
        in_names, out_names, out_avals, zero_outs = [], [], [], []
        for alloc in nc.m.functions[0].allocations:
            if not isinstance(alloc, mybir.MemoryLocationSet):
                continue
            name = alloc.memorylocations[0].name
            if alloc.kind == "ExternalInput":
                if name != partition_name:
                    in_names.append(name)
            elif alloc.kind == "ExternalOutput":
                out_names.append(name)
                shape = tuple(alloc.tensor_shape)
                dtype = mybir.dt.np(alloc.dtype)
                out_avals.append(jax.core.ShapedArray(shape, dtype))
                zero_outs.append(np.zeros(shape, dtype))
        self.in_names, self.out_names = in_names, out_names
        n_params, n_outs = len(in_names), len(out_avals)
        all_names = in_names + out_names + (
            [partition_name] if partition_name else []
        )

        def _body(*args):
            operands = list(args)
            if partition_name is not None:
                operands.append(partition_id_tensor())
            outs = _bass_exec_p.bind(
                *operands,
                out_avals=tuple(out_avals),
                in_names=tuple(all_names),
                out_names=tuple(out_names),
                lowering_input_output_aliases=(),
                sim_require_finite=True,
                sim_require_nnan=True,
                nc=nc,
            )
            return tuple(outs)

        devices = jax.devices()[:N_CORES]
        mesh = Mesh(np.asarray(devices), ("core",))
        self.sharded = jax.jit(
            shard_map(
                _body,
                mesh=mesh,
                in_specs=(PartitionSpec("core"),) * (n_params + n_outs),
                out_specs=(PartitionSpec("core"),) * n_outs,
                check_rep=False,
            ),
            keep_unused=True,
        )
        self.sharding = NamedSharding(mesh, PartitionSpec("core"))
        self.zero_outs = zero_outs

    def stage(self, in_maps):
        """device_put concatenated per-core inputs + zero outputs once."""
        jax = self.jax
        concat = [
            np.concatenate([np.asarray(m[nm]) for m in in_maps], axis=0)
            for nm in self.in_names
        ]
        concat += [
            np.concatenate([z] * len(in_maps), axis=0) for z in self.zero_outs
        ]
        self.dev_args = [jax.device_put(a, self.sharding) for a in concat]
        jax.block_until_ready(self.dev_args)

    def run(self):
        out = self.sharded(*self.dev_args)
        self.jax.block_until_ready(out)
        return out


def _bf16(x):
    return np.ascontiguousarray(x, dtype=ml_dtypes.bfloat16)


def make_in_maps(
    query,
    key_,
    value,
    in_proj_weight,
    in_proj_bias,
    out_proj_bias,
    out_proj_weight=None,
    **kw,
):
    if out_proj_weight is None:
        out_proj_weight = kw["out_proj_weight"]
    query = np.asarray(query, np.float32)
    key_ = np.asarray(key_, np.float32)
    value = np.asarray(value, np.float32)
    W = np.asarray(in_proj_weight, np.float32)
    Wo = np.asarray(out_proj_weight, np.float32)
    bi = np.asarray(in_proj_bias, np.float32)
    assert not np.any(bi), "nonzero in_proj_bias unsupported by this kernel"

    scale = np.float32(1.0 / np.sqrt(D))
    wq, wk, wv = W[:E], W[E : 2 * E], W[2 * E :]

    # (E, BT) b-major transposed activations
    XTq = np.ascontiguousarray(query.transpose(2, 1, 0).reshape(E, TB))
    XTk = np.ascontiguousarray(key_.transpose(2, 1, 0).reshape(E, TB))
    XTv = np.ascontiguousarray(value.transpose(2, 1, 0).reshape(E, TB))

    mask = np.triu(np.ones((C, C), np.float32))  # U[s,t]=1 iff s<=t

    in_maps = []
    for core in range(N_CORES):
        hg, g = divmod(core, TBG)
        cols = np.r_[g * 256 : (g + 1) * 256, T + g * 256 : T + (g + 1) * 256]
        hsl = slice(hg * DHC, (hg + 1) * DHC)
        coefs = np.zeros((128, N_CORES), np.float32)
        for cid in range(N_CORES):
            if cid // TBG == hg and cid % TBG < g:
                coefs[:, cid] = 1.0
        in_maps.append(
            {
                "xtq": _bf16(XTq[:, cols]),
                "xtk": _bf16(XTk[:, cols]),
                "xtv": _bf16(XTv[:, cols]),
                "wqT": _bf16((wq[hsl, :] * scale).T),
                "wkT": _bf16(wk[hsl, :].T),
                "wvT": _bf16(wv[hsl, :].T),
                "woT": _bf16(Wo[:, hsl].T.copy()),
                "maskd": mask,
                "coefsd": coefs,
            }
        )
    return in_maps


def assemble_output(results, bo):
    out = np.zeros((T, B, E), np.float32)
    for core in range(N_CORES):
        hg, g = divmod(core, TBG)
        po = results[core]["pout"]  # (512, 1024) rows: b*256 + tl
        for b in range(B):
            out[g * 256 : (g + 1) * 256, b, :] += po[b * 256 : (b + 1) * 256, :]
    out += bo
    return out


def kernel(**inputs):
    in_maps = make_in_maps(**inputs)
    bo = np.asarray(inputs["out_proj_bias"], np.float32)
    nc = _build_nc()
    res = run_bass_kernel_spmd(nc, in_maps, list(range(N_CORES)))
    return assemble_output(res.results, bo)


# revision 19
# speedup vs baseline: 12001.7726x; 259.4313x over previous
"""Cumulative linear multihead attention (KV prefix-scan) on 8 TRN2 NeuronCores.

Sharding: 4 sequence(tb)-groups x 2 head-groups. Core c = hg*4 + g handles
t-range [g*256,(g+1)*256) for both batches and heads [hg*8, hg*8+8).
Per core: column-parallel in_proj for its heads over its tb rows, chunked
linear attention (chunk=128) with the cross-core KV prefix state exchanged
via an 8-core AllGather, then a row/column partial out_proj. Host sums the
two head-group partials per tb row.
"""
import numpy as np
import ml_dtypes

import concourse.bass as bass
import concourse.mybir as mybir
import concourse.tile as tile
from concourse.bass_utils import run_bass_kernel_spmd

T, B, E, H, D = 1024, 2, 1024, 16, 64
TB = T * B
N_CORES = 8
TBG = 4        # tb groups
HGS = 2        # head groups
TBC = TB // TBG          # 512 tb rows per core
DHC = (H // HGS) * D     # 512 head dims per core per projection
NP = (H // HGS) * B      # 16 (b,h) pairs per core
C = 128                  # chunk
NCH = TBC // (B * C)     # 2 chunks per (b,h) per core
BF = mybir.dt.bfloat16
F32 = mybir.dt.float32


_MAXW = 1  # this walrus build allows a single sync-wait condition per instruction


def _split_excess_waits(nc):
    """Hoist sync waits beyond _MAXW onto same-engine NOPs placed just before
    the over-constrained instruction (engine streams execute in list order)."""
    n_spliced = 0
    for fn in nc.m.functions:
        for bb in fn.blocks:
            insts = bb.instructions
            i = 0
            while i < len(insts):
                ins = insts[i]
                si = getattr(ins, "sync_info", None)
                if si is not None and len(si.on_wait) > _MAXW:
                    waits = list(si.on_wait)
                    keep = waits[-_MAXW:]
                    extra = waits[:-_MAXW]
                    for j in range(0, len(extra), _MAXW):
                        nop = mybir.InstNoOp(
                            name=f"waitsplit_{n_spliced}",
                            engine=ins.engine,
                            bass_nofuse=True,
                            sync_info=mybir.SyncInfo(
                                on_wait=extra[j : j + _MAXW], on_update=[]
                            ),
                        )
                        insts.insert(i, nop)
                        i += 1
                        n_spliced += 1
                    ins.sync_info = mybir.SyncInfo(
                        on_wait=keep, on_update=list(si.on_update)
                    )
                i += 1
    return n_spliced


_NC_CACHE = {}


def _build_nc(reps: int = 1):
    if reps in _NC_CACHE:
        return _NC_CACHE[reps]
    nc = bass.Bass()
    xtq = nc.dram_tensor("xtq", [E, TBC], BF, kind="ExternalInput")
    xtk = nc.dram_tensor("xtk", [E, TBC], BF, kind="ExternalInput")
    xtv = nc.dram_tensor("xtv", [E, TBC], BF, kind="ExternalInput")
    wqT = nc.dram_tensor("wqT", [E, DHC], BF, kind="ExternalInput")
    wkT = nc.dram_tensor("wkT", [E, DHC], BF, kind="ExternalInput")
    wvT = nc.dram_tensor("wvT", [E, DHC], BF, kind="ExternalInput")
    woT = nc.dram_tensor("woT", [DHC, E], BF, kind="ExternalInput")
    maskd = nc.dram_tensor("maskd", [C, C], F32, kind="ExternalInput")
    coefsd = nc.dram_tensor("coefsd", [128, N_CORES], F32, kind="ExternalInput")
    pout = nc.dram_tensor("pout", [TBC, E], F32, kind="ExternalOutput")
    cc_in = nc.dram_tensor("cc_in", [D, NP * D], BF)
    cc_shared = nc.dram_tensor(
        "cc_shared", [N_CORES * D, NP * D], BF, addr_space="Shared"
    )

    mult = mybir.AluOpType.mult

    with tile.TileContext(nc) as tc:
        with (
            tc.tile_pool(name="wpool", bufs=1) as wpool,
            tc.tile_pool(name="actpool", bufs=1) as actpool,
            tc.tile_pool(name="stpool", bufs=1) as stpool,
            tc.tile_pool(name="ampool", bufs=1) as ampool,
            tc.tile_pool(name="obuf", bufs=3) as obuf,
            tc.tile_pool(name="ps_big", bufs=2, space="PSUM") as ps_big,
            tc.tile_pool(name="ps_kv", bufs=2, space="PSUM") as ps_kv,
            tc.tile_pool(name="ps_at", bufs=2, space="PSUM") as ps_at,
            tc.tile_pool(name="ps_io", bufs=2, space="PSUM") as ps_io,
        ):
          for _rep in range(reps):
            def load_tiles(src, n, w, nm):
                ts = []
                for k in range(n):
                    t = wpool.tile([128, w], BF, name=f"{nm}{k}")
                    nc.sync.dma_start(out=t[:], in_=src[k * 128 : (k + 1) * 128, :])
                    ts.append(t)
                return ts

            # k/v-side loads first: they gate the L states -> exchange
            xk_sb = load_tiles(xtk, 8, TBC, "xk")
            xv_sb = load_tiles(xtv, 8, TBC, "xv")
            wk_sb = load_tiles(wkT, 8, DHC, "wk")
            wv_sb = load_tiles(wvT, 8, DHC, "wv")

            def proj_rows(x_tiles, w_tiles, nm):
                outs = []
                for i in range(4):
                    ps = ps_big.tile([128, DHC], F32, name="ps_proj")
                    for k in range(8):
                        nc.tensor.matmul(
                            ps[:],
                            lhsT=x_tiles[k][:, i * 128 : (i + 1) * 128],
                            rhs=w_tiles[k][:],
                            start=(k == 0),
                            stop=(k == 7),
                        )
                    o = actpool.tile([128, DHC], BF, name=f"{nm}{i}")
                    nc.vector.tensor_copy(out=o[:], in_=ps[:])
                    outs.append(o)
                return outs

            def proj_cols(x_tiles, w_tiles, nm):
                outs = []
                for j in range(4):
                    ps = ps_big.tile([128, TBC], F32, name="ps_proj")
                    for k in range(8):
                        nc.tensor.matmul(
                            ps[:],
                            lhsT=w_tiles[k][:, j * 128 : (j + 1) * 128],
                            rhs=x_tiles[k][:],
                            start=(k == 0),
                            stop=(k == 7),
                        )
                    o = actpool.tile([128, TBC], BF, name=f"{nm}{j}")
                    nc.vector.tensor_copy(out=o[:], in_=ps[:])
                    outs.append(o)
                return outs

            kS_sb = proj_rows(xk_sb, wk_sb, "kS")
            v_sb = proj_rows(xv_sb, wv_sb, "v")

            # ---- local KV chunk states ----
            kv0_all = stpool.tile([D, NP * D], F32, name="kv0_all")
            kv1_all = stpool.tile([D, NP * D], F32, name="kv1_all")
            for b in range(B):
                for c in range(NCH):
                    it = b * 2 + c
                    ps = ps_kv.tile([D, 8 * D], F32, name="ps_kv")
                    for h in range(8):
                        nc.tensor.matmul(
                            ps[:, h * D : (h + 1) * D],
                            lhsT=kS_sb[it][:, h * D : (h + 1) * D],
                            rhs=v_sb[it][:, h * D : (h + 1) * D],
                            start=True,
                            stop=True,
                        )
                    dst = kv0_all if c == 0 else kv1_all
                    nc.vector.tensor_copy(
                        out=dst[:, b * 8 * D : (b + 1) * 8 * D], in_=ps[:]
                    )
            l_bf = stpool.tile([D, NP * D], BF, name="l_bf")
            nc.vector.tensor_add(out=l_bf[:], in0=kv0_all[:], in1=kv1_all[:])

            # ---- exchange: bf16 L-state allgather ----
            nc.sync.dma_start(out=cc_in[:], in_=l_bf[:])
            nc.gpsimd.collective_compute(
                "AllGather",
                mybir.AluOpType.bypass,
                replica_groups=[list(range(N_CORES))],
                ins=[cc_in[:]],
                outs=[cc_shared[:]],
            )

            # remaining inputs (loads overlap L/exchange)
            xq_sb = load_tiles(xtq, 8, TBC, "xq")
            wq_sb = load_tiles(wqT, 8, DHC, "wq")
            wo_sb = load_tiles(woT, 4, E, "wo")
            mask_sb = wpool.tile([C, C], F32, name="mask_sb")
            nc.sync.dma_start(out=mask_sb[:], in_=maskd[:])
            coefs_sb = wpool.tile([128, N_CORES], F32, name="coefs_sb")
            nc.sync.dma_start(out=coefs_sb[:], in_=coefsd[:])

            qT_sb = proj_cols(xq_sb, wq_sb, "qT")
            kT_sb = proj_cols(xk_sb, wk_sb, "kT")

            # ---- A^T + mask ----
            am_sb = {}
            for p in range(NP):
                b, h = divmod(p, NP // B)
                jj, ro = divmod(h, 2)
                ro *= D
                for c in range(NCH):
                    col = b * 256 + c * 128
                    ps = ps_at.tile([C, C], F32, name="ps_at")
                    nc.tensor.matmul(
                        ps[:],
                        lhsT=kT_sb[jj][ro : ro + D, col : col + C],
                        rhs=qT_sb[jj][ro : ro + D, col : col + C],
                        start=True,
                        stop=True,
                    )
                    am = ampool.tile([C, C], BF, name=f"am{p}_{c}")
                    nc.vector.tensor_tensor(
                        out=am[:], in0=ps[:], in1=mask_sb[:], op=mult
                    )
                    am_sb[(p, c)] = am

            # ---- read slots (after barrier), cast to f32 via gpsimd DMA ----
            cc_sb = []
            for i in range(N_CORES):
                t = stpool.tile([D, NP * D], F32, name=f"cc{i}")
                nc.gpsimd.dma_start(
                    out=t[:], in_=cc_shared[i * D : (i + 1) * D, :]
                )
                cc_sb.append(t)
            pcur = stpool.tile([D, NP * D], F32, name="pfx0")
            nc.vector.memset(pcur[:], 0.0)
            for cid in range(N_CORES):
                pnxt = stpool.tile([D, NP * D], F32, name=f"pfx{cid+1}")
                nc.vector.scalar_tensor_tensor(
                    out=pnxt[:],
                    in0=cc_sb[cid][:],
                    scalar=coefs_sb[0:D, cid : cid + 1],
                    in1=pcur[:],
                    op0=mult,
                    op1=mybir.AluOpType.add,
                )
                pcur = pnxt
            s1f = stpool.tile([D, NP * D], F32, name="s1f")
            nc.vector.tensor_add(out=s1f[:], in0=pcur[:], in1=kv0_all[:])
            s0b = stpool.tile([128, NP * D], BF, name="s0b")
            s1b = stpool.tile([128, NP * D], BF, name="s1b")
            nc.vector.tensor_copy(out=s0b[0:D, :], in_=pcur[:])
            nc.vector.tensor_copy(out=s0b[D : 2 * D, :], in_=pcur[:])
            nc.vector.tensor_copy(out=s1b[0:D, :], in_=s1f[:])
            nc.vector.tensor_copy(out=s1b[D : 2 * D, :], in_=s1f[:])

            # ---- intra + inter -> outT ----
            outT_sb = {
                (j, i): actpool.tile([128, 128], BF, name=f"outT{j}_{i}")
                for j in range(4)
                for i in range(4)
            }
            for p in range(NP):
                b, h = divmod(p, NP // B)
                jj, ro = divmod(h, 2)
                ro *= D
                for c in range(NCH):
                    it = b * 2 + c
                    col = b * 256 + c * 128
                    ps = ps_io.tile([D, C], F32, name="ps_io")
                    nc.tensor.matmul(
                        ps[:],
                        lhsT=v_sb[it][:, h * D : (h + 1) * D],
                        rhs=am_sb[(p, c)][:],
                        start=True,
                        stop=False,
                    )
                    sb = s0b if c == 0 else s1b
                    nc.tensor.matmul(
                        ps[:],
                        lhsT=sb[ro : ro + D, p * D : (p + 1) * D],
                        rhs=qT_sb[jj][ro : ro + D, col : col + C],
                        start=False,
                        stop=True,
                    )
                    nc.vector.tensor_copy(
                        out=outT_sb[(jj, col // 128)][ro : ro + D, :], in_=ps[:]
                    )

            # ---- out_proj partial ----
            for i in range(4):
                for n in range(2):
                    ps = ps_big.tile([128, 512], F32, name="ps_proj")
                    for k in range(4):
                        nc.tensor.matmul(
                            ps[:],
                            lhsT=outT_sb[(k, i)][:, :],
                            rhs=wo_sb[k][:, n * 512 : (n + 1) * 512],
                            start=(k == 0),
                            stop=(k == 3),
                        )
                    ob = obuf.tile([128, 512], F32, name="ob")
                    nc.vector.tensor_copy(out=ob[:], in_=ps[:])
                    nc.sync.dma_start(
                        out=pout[i * 128 : (i + 1) * 128, n * 512 : (n + 1) * 512],
                        in_=ob[:],
                    )
    _split_excess_waits(nc)
    _NC_CACHE[reps] = nc
    return nc


class _Runner:
    """Compile a Bass module once via bass2jax/PJRT and allow repeated
    executions with device-resident inputs (no donation, no per-call
    retrace). Used by test.py for HW timing."""

    def __init__(self, nc):
        import jax
        from jax.sharding import Mesh, PartitionSpec, NamedSharding
        from jax.experimental.shard_map import shard_map
        from concourse.bass2jax import (
            _bass_exec_p,
            partition_id_tensor,
            install_neuronx_cc_hook,
        )

        install_neuronx_cc_hook()
        self.jax = jax
        partition_name = (
            nc.partition_id_tensor.name if nc.partition_id_tensor else None
        )
        in_names, out_names, out_avals, zero_outs = [], [], [], []
        for alloc in nc.m.functions[0].allocations:
            if not isinstance(alloc, mybir.MemoryLocationSet):
                continue
            name = alloc.memorylocations[0].name
            if alloc.kind == "ExternalInput":
                if name != partition_name:
                    in_names.append(name)
            elif alloc.kind == "ExternalOutput":
                out_names.append(name)
                shape = tuple(alloc.tensor_shape)
                dtype = mybir.dt.np(alloc.dtype)
                out_avals.append(jax.core.ShapedArray(shape, dtype))
                zero_outs.append(np.zeros(shape, dtype))
        self.in_names, self.out_names = in_names, out_names
        n_params, n_outs = len(in_names), len(out_avals)
        all_names = in_names + out_names + (
            [partition_name] if partition_name else []
        )

        def _body(*args):
            operands = list(args)
            if partition_name is not None:
                operands.append(partition_id_tensor())
            outs = _bass_exec_p.bind(
                *operands,
                out_avals=tuple(out_avals),
                in_names=tuple(all_names),
                out_names=tuple(out_names),
                lowering_input_output_aliases=(),
                sim_require_finite=True,
                sim_require_nnan=True,
                nc=nc,
            )
            return tuple(outs)

        devices = jax.devices()[:N_CORES]
        mesh = Mesh(np.asarray(devices), ("core",))
        self.sharded = jax.jit(
            shard_map(
                _body,
                mesh=mesh,
                in_specs=(PartitionSpec("core"),) * (n_params + n_outs),
                out_specs=(PartitionSpec("core"),) * n_outs,
                check_rep=False,
            ),
            keep_unused=True,
        )
        self.sharding = NamedSharding(mesh, PartitionSpec("core"))
        self.zero_outs = zero_outs

    def stage(self, in_maps):
        """device_put concatenated per-core inputs + zero outputs once."""
        jax = self.jax
        concat = [
            np.concatenate([np.asarray(m[nm]) for m in in_maps], axis=0)
            for nm in self.in_names
        ]
        concat += [
            np.concatenate([z] * len(in_maps), axis=0) for z in self.zero_outs
        ]
        self.dev_args = [jax.device_put(a, self.sharding) for a in concat]
        jax.block_until_ready(self.dev_args)

    def run(self):
        out = self.sharded(*self.dev_args)
        self.jax.block_until_ready(out)
        return out


def _bf16(x):
    return np.ascontiguousarray(x, dtype=ml_dtypes.bfloat16)


def make_in_maps(
    query,
    key_,
    value,
    in_proj_weight,
    in_proj_bias,
    out_proj_bias,
    out_proj_weight=None,
    **kw,
):
    if out_proj_weight is None:
        out_proj_weight = kw["out_proj_weight"]
    query = np.asarray(query, np.float32)
    key_ = np.asarray(key_, np.float32)
    value = np.asarray(value, np.float32)
    W = np.asarray(in_proj_weight, np.float32)
    Wo = np.asarray(out_proj_weight, np.float32)
    bi = np.asarray(in_proj_bias, np.float32)
    assert not np.any(bi), "nonzero in_proj_bias unsupported by this kernel"

    scale = np.float32(1.0 / np.sqrt(D))
    wq, wk, wv = W[:E], W[E : 2 * E], W[2 * E :]

    # (E, BT) b-major transposed activations
    XTq = np.ascontiguousarray(query.transpose(2, 1, 0).reshape(E, TB))
    XTk = np.ascontiguousarray(key_.transpose(2, 1, 0).reshape(E, TB))
    XTv = np.ascontiguousarray(value.transpose(2, 1, 0).reshape(E, TB))

    mask = np.triu(np.ones((C, C), np.float32))  # U[s,t]=1 iff s<=t

    in_maps = []
    for core in range(N_CORES):
        hg, g = divmod(core, TBG)
        cols = np.r_[g * 256 : (g + 1) * 256, T + g * 256 : T + (g + 1) * 256]
        hsl = slice(hg * DHC, (hg + 1) * DHC)
        coefs = np.zeros((128, N_CORES), np.float32)
        for cid in range(N_CORES):
            if cid // TBG == hg and cid % TBG < g:
                coefs[:, cid] = 1.0
        in_maps.append(
            {
                "xtq": _bf16(XTq[:, cols]),
                "xtk": _bf16(XTk[:, cols]),
                "xtv": _bf16(XTv[:, cols]),
                "wqT": _bf16((wq[hsl, :] * scale).T),
                "wkT": _bf16(wk[hsl, :].T),
                "wvT": _bf16(wv[hsl, :].T),
                "woT": _bf16(Wo[:, hsl].T.copy()),
                "maskd": mask,
                "coefsd": coefs,
            }
        )
    return in_maps


def assemble_output(results, bo):
    out = np.zeros((T, B, E), np.float32)
    for core in range(N_CORES):
        hg, g = divmod(core, TBG)
        po = results[core]["pout"]  # (512, 1024) rows: b*256 + tl
        for b in range(B):
            out[g * 256 : (g + 1) * 256, b, :] += po[b * 256 : (b + 1) * 256, :]
    out += bo
    return out


def kernel(**inputs):
    in_maps = make_in_maps(**inputs)
    bo = np.asarray(inputs["out_proj_bias"], np.float32)
    nc = _build_nc()
    res = run_bass_kernel_spmd(nc, in_maps, list(range(N_CORES)))
    return assemble_output(res.results, bo)


# revision 20
# speedup vs baseline: 14295.1343x; 1.1911x over previous
"""Cumulative linear multihead attention (KV prefix-scan) on 8 TRN2 NeuronCores.

Sharding: 4 sequence(tb)-groups x 2 head-groups. Core c = hg*4 + g handles
t-range [g*256,(g+1)*256) for both batches and heads [hg*8, hg*8+8).
Per core: column-parallel in_proj for its heads over its tb rows, chunked
linear attention (chunk=128) with the cross-core KV prefix state exchanged
via an 8-core AllGather, then a row/column partial out_proj. Host sums the
two head-group partials per tb row.
"""
import numpy as np
import ml_dtypes

import concourse.bass as bass
import concourse.mybir as mybir
import concourse.tile as tile
from concourse.bass_utils import run_bass_kernel_spmd

T, B, E, H, D = 1024, 2, 1024, 16, 64
TB = T * B
N_CORES = 8
TBG = 4        # tb groups
HGS = 2        # head groups
TBC = TB // TBG          # 512 tb rows per core
DHC = (H // HGS) * D     # 512 head dims per core per projection
NP = (H // HGS) * B      # 16 (b,h) pairs per core
C = 128                  # chunk
NCH = TBC // (B * C)     # 2 chunks per (b,h) per core
BF = mybir.dt.bfloat16
F32 = mybir.dt.float32


_MAXW = 1  # this walrus build allows a single sync-wait condition per instruction


def _split_excess_waits(nc):
    """Hoist sync waits beyond _MAXW onto same-engine NOPs placed just before
    the over-constrained instruction (engine streams execute in list order)."""
    n_spliced = 0
    for fn in nc.m.functions:
        for bb in fn.blocks:
            insts = bb.instructions
            i = 0
            while i < len(insts):
                ins = insts[i]
                si = getattr(ins, "sync_info", None)
                if si is not None and len(si.on_wait) > _MAXW:
                    waits = list(si.on_wait)
                    keep = waits[-_MAXW:]
                    extra = waits[:-_MAXW]
                    for j in range(0, len(extra), _MAXW):
                        nop = mybir.InstNoOp(
                            name=f"waitsplit_{n_spliced}",
                            engine=ins.engine,
                            bass_nofuse=True,
                            sync_info=mybir.SyncInfo(
                                on_wait=extra[j : j + _MAXW], on_update=[]
                            ),
                        )
                        insts.insert(i, nop)
                        i += 1
                        n_spliced += 1
                    ins.sync_info = mybir.SyncInfo(
                        on_wait=keep, on_update=list(si.on_update)
                    )
                i += 1
    return n_spliced


_NC_CACHE = {}


def _build_nc(reps: int = 1):
    if reps in _NC_CACHE:
        return _NC_CACHE[reps]
    nc = bass.Bass()
    xtq = nc.dram_tensor("xtq", [E, TBC], BF, kind="ExternalInput")
    xtk = nc.dram_tensor("xtk", [E, TBC], BF, kind="ExternalInput")
    xtv = nc.dram_tensor("xtv", [E, TBC], BF, kind="ExternalInput")
    wqT = nc.dram_tensor("wqT", [E, DHC], BF, kind="ExternalInput")
    wkT = nc.dram_tensor("wkT", [E, DHC], BF, kind="ExternalInput")
    wvT = nc.dram_tensor("wvT", [E, DHC], BF, kind="ExternalInput")
    woT = nc.dram_tensor("woT", [DHC, E], BF, kind="ExternalInput")
    maskd = nc.dram_tensor("maskd", [C, C], F32, kind="ExternalInput")
    coefsd = nc.dram_tensor("coefsd", [128, N_CORES], F32, kind="ExternalInput")
    pout = nc.dram_tensor("pout", [TBC, E], F32, kind="ExternalOutput")
    cc_in = nc.dram_tensor("cc_in", [D, NP * D], BF)
    cc_shared = nc.dram_tensor(
        "cc_shared", [N_CORES * D, NP * D], BF, addr_space="Shared"
    )

    mult = mybir.AluOpType.mult

    with tile.TileContext(nc) as tc:
        with (
            tc.tile_pool(name="wpool", bufs=1) as wpool,
            tc.tile_pool(name="actpool", bufs=1) as actpool,
            tc.tile_pool(name="stpool", bufs=1) as stpool,
            tc.tile_pool(name="ampool", bufs=1) as ampool,
            tc.tile_pool(name="obuf", bufs=3) as obuf,
            tc.tile_pool(name="ps_big", bufs=2, space="PSUM") as ps_big,
            tc.tile_pool(name="ps_kv", bufs=2, space="PSUM") as ps_kv,
            tc.tile_pool(name="ps_at", bufs=2, space="PSUM") as ps_at,
            tc.tile_pool(name="ps_io", bufs=2, space="PSUM") as ps_io,
        ):
          for _rep in range(reps):
            def load_tiles(src, n, w, nm):
                ts = []
                for k in range(n):
                    t = wpool.tile([128, w], BF, name=f"{nm}{k}")
                    nc.sync.dma_start(out=t[:], in_=src[k * 128 : (k + 1) * 128, :])
                    ts.append(t)
                return ts

            # k/v-side loads first: they gate the L states -> exchange
            xk_sb = load_tiles(xtk, 8, TBC, "xk")
            xv_sb = load_tiles(xtv, 8, TBC, "xv")
            wk_sb = load_tiles(wkT, 8, DHC, "wk")
            wv_sb = load_tiles(wvT, 8, DHC, "wv")

            def proj_rows(x_tiles, w_tiles, nm):
                outs = []
                for i in range(4):
                    ps = ps_big.tile([128, DHC], F32, name="ps_proj")
                    for k in range(8):
                        nc.tensor.matmul(
                            ps[:],
                            lhsT=x_tiles[k][:, i * 128 : (i + 1) * 128],
                            rhs=w_tiles[k][:],
                            start=(k == 0),
                            stop=(k == 7),
                        )
                    o = actpool.tile([128, DHC], BF, name=f"{nm}{i}")
                    nc.scalar.copy(out=o[:], in_=ps[:])
                    outs.append(o)
                return outs

            def proj_cols(x_tiles, w_tiles, nm):
                outs = []
                for j in range(4):
                    ps = ps_big.tile([128, TBC], F32, name="ps_proj")
                    for k in range(8):
                        nc.tensor.matmul(
                            ps[:],
                            lhsT=w_tiles[k][:, j * 128 : (j + 1) * 128],
                            rhs=x_tiles[k][:],
                            start=(k == 0),
                            stop=(k == 7),
                        )
                    o = actpool.tile([128, TBC], BF, name=f"{nm}{j}")
                    nc.scalar.copy(out=o[:], in_=ps[:])
                    outs.append(o)
                return outs

            kS_sb = proj_rows(xk_sb, wk_sb, "kS")
            v_sb = proj_rows(xv_sb, wv_sb, "v")

            # ---- local KV chunk states ----
            kv0_all = stpool.tile([D, NP * D], F32, name="kv0_all")
            kv1_all = stpool.tile([D, NP * D], F32, name="kv1_all")
            for b in range(B):
                for c in range(NCH):
                    it = b * 2 + c
                    ps = ps_kv.tile([D, 8 * D], F32, name="ps_kv")
                    for h in range(8):
                        nc.tensor.matmul(
                            ps[:, h * D : (h + 1) * D],
                            lhsT=kS_sb[it][:, h * D : (h + 1) * D],
                            rhs=v_sb[it][:, h * D : (h + 1) * D],
                            start=True,
                            stop=True,
                        )
                    dst = kv0_all if c == 0 else kv1_all
                    nc.scalar.copy(
                        out=dst[:, b * 8 * D : (b + 1) * 8 * D], in_=ps[:]
                    )
            l_bf = stpool.tile([D, NP * D], BF, name="l_bf")
            nc.vector.tensor_add(out=l_bf[:], in0=kv0_all[:], in1=kv1_all[:])

            # ---- exchange: bf16 L-state allgather ----
            nc.sync.dma_start(out=cc_in[:], in_=l_bf[:])
            nc.gpsimd.collective_compute(
                "AllGather",
                mybir.AluOpType.bypass,
                replica_groups=[list(range(N_CORES))],
                ins=[cc_in[:]],
                outs=[cc_shared[:]],
            )

            # remaining inputs (loads overlap L/exchange)
            xq_sb = load_tiles(xtq, 8, TBC, "xq")
            wq_sb = load_tiles(wqT, 8, DHC, "wq")
            wo_sb = load_tiles(woT, 4, E, "wo")
            mask_sb = wpool.tile([C, C], F32, name="mask_sb")
            nc.sync.dma_start(out=mask_sb[:], in_=maskd[:])
            coefs_sb = wpool.tile([128, N_CORES], F32, name="coefs_sb")
            nc.sync.dma_start(out=coefs_sb[:], in_=coefsd[:])

            qT_sb = proj_cols(xq_sb, wq_sb, "qT")
            kT_sb = proj_cols(xk_sb, wk_sb, "kT")

            # ---- A^T + mask ----
            am_sb = {}
            for p in range(NP):
                b, h = divmod(p, NP // B)
                jj, ro = divmod(h, 2)
                ro *= D
                for c in range(NCH):
                    col = b * 256 + c * 128
                    ps = ps_at.tile([C, C], F32, name="ps_at")
                    nc.tensor.matmul(
                        ps[:],
                        lhsT=kT_sb[jj][ro : ro + D, col : col + C],
                        rhs=qT_sb[jj][ro : ro + D, col : col + C],
                        start=True,
                        stop=True,
                    )
                    am = ampool.tile([C, C], BF, name=f"am{p}_{c}")
                    nc.vector.tensor_tensor(
                        out=am[:], in0=ps[:], in1=mask_sb[:], op=mult
                    )
                    am_sb[(p, c)] = am

            # ---- read slots (after barrier), cast to f32 via gpsimd DMA ----
            cc_sb = []
            for i in range(N_CORES):
                t = stpool.tile([D, NP * D], F32, name=f"cc{i}")
                nc.gpsimd.dma_start(
                    out=t[:], in_=cc_shared[i * D : (i + 1) * D, :]
                )
                cc_sb.append(t)
            pcur = stpool.tile([D, NP * D], F32, name="pfx0")
            nc.vector.memset(pcur[:], 0.0)
            for cid in range(N_CORES):
                pnxt = stpool.tile([D, NP * D], F32, name=f"pfx{cid+1}")
                nc.vector.scalar_tensor_tensor(
                    out=pnxt[:],
                    in0=cc_sb[cid][:],
                    scalar=coefs_sb[0:D, cid : cid + 1],
                    in1=pcur[:],
                    op0=mult,
                    op1=mybir.AluOpType.add,
                )
                pcur = pnxt
            s1f = stpool.tile([D, NP * D], F32, name="s1f")
            nc.vector.tensor_add(out=s1f[:], in0=pcur[:], in1=kv0_all[:])
            s0b = stpool.tile([128, NP * D], BF, name="s0b")
            s1b = stpool.tile([128, NP * D], BF, name="s1b")
            nc.vector.tensor_copy(out=s0b[0:D, :], in_=pcur[:])
            nc.vector.tensor_copy(out=s0b[D : 2 * D, :], in_=pcur[:])
            nc.vector.tensor_copy(out=s1b[0:D, :], in_=s1f[:])
            nc.vector.tensor_copy(out=s1b[D : 2 * D, :], in_=s1f[:])

            # ---- intra + inter -> outT ----
            outT_sb = {
                (j, i): actpool.tile([128, 128], BF, name=f"outT{j}_{i}")
                for j in range(4)
                for i in range(4)
            }
            for p in range(NP):
                b, h = divmod(p, NP // B)
                jj, ro = divmod(h, 2)
                ro *= D
                for c in range(NCH):
                    it = b * 2 + c
                    col = b * 256 + c * 128
                    ps = ps_io.tile([D, C], F32, name="ps_io")
                    nc.tensor.matmul(
                        ps[:],
                        lhsT=v_sb[it][:, h * D : (h + 1) * D],
                        rhs=am_sb[(p, c)][:],
                        start=True,
                        stop=False,
                    )
                    sb = s0b if c == 0 else s1b
                    nc.tensor.matmul(
                        ps[:],
                        lhsT=sb[ro : ro + D, p * D : (p + 1) * D],
                        rhs=qT_sb[jj][ro : ro + D, col : col + C],
                        start=False,
                        stop=True,
                    )
                    nc.scalar.copy(
                        out=outT_sb[(jj, col // 128)][ro : ro + D, :], in_=ps[:]
                    )

            # ---- out_proj partial ----
            for i in range(4):
                for n in range(2):
                    ps = ps_big.tile([128, 512], F32, name="ps_proj")
                    for k in range(4):
                        nc.tensor.matmul(
                            ps[:],
                            lhsT=outT_sb[(k, i)][:, :],
                            rhs=wo_sb[k][:, n * 512 : (n + 1) * 512],
                            start=(k == 0),
                            stop=(k == 3),
                        )
                    ob = obuf.tile([128, 512], F32, name="ob")
                    nc.scalar.copy(out=ob[:], in_=ps[:])
                    nc.sync.dma_start(
                        out=pout[i * 128 : (i + 1) * 128, n * 512 : (n + 1) * 512],
                        in_=ob[:],
                    )
    _split_excess_waits(nc)
    _NC_CACHE[reps] = nc
    return nc


class _Runner:
    """Compile a Bass module once via bass2jax/PJRT and allow repeated
    executions with device-resident inputs (no donation, no per-call
    retrace). Used by test.py for HW timing."""

    def __init__(self, nc):
        import jax
        from jax.sharding import Mesh, PartitionSpec, NamedSharding
        from jax.experimental.shard_map import shard_map
        from concourse.bass2jax import (
            _bass_exec_p,
            partition_id_tensor,
            install_neuronx_cc_hook,
        )

        install_neuronx_cc_hook()
        self.jax = jax
        partition_name = (
            nc.partition_id_tensor.name if nc.partition_id_tensor else None
        )
        in_names, out_names, out_avals, zero_outs = [], [], [], []
        for alloc in nc.m.functions[0].allocations:
            if not isinstance(alloc, mybir.MemoryLocationSet):
                continue
            name = alloc.memorylocations[0].name
            if alloc.kind == "ExternalInput":
                if name != partition_name:
                    in_names.append(name)
            elif alloc.kind == "ExternalOutput":
                out_names.append(name)
                shape = tuple(alloc.tensor_shape)
                dtype = mybir.dt.np(alloc.dtype)
                out_avals.append(jax.core.ShapedArray(shape, dtype))
                zero_outs.append(np.zeros(shape, dtype))
        self.in_names, self.out_names = in_names, out_names
        n_params, n_outs = len(in_names), len(out_avals)
        all_names = in_names + out_names + (
            [partition_name] if partition_name else []
        )

        def _body(*args):
            operands = list(args)
            if partition_name is not None:
                operands.append(partition_id_tensor())
            outs = _bass_exec_p.bind(
                *operands,
                out_avals=tuple(out_avals),
                in_names=tuple(all_names),
                out_names=tuple(out_names),
                lowering_input_output_aliases=(),
                sim_require_finite=True,
                sim_require_nnan=True,
                nc=nc,
            )
            return tuple(outs)

        devices = jax.devices()[:N_CORES]
        mesh = Mesh(np.asarray(devices), ("core",))
        self.sharded = jax.jit(
            shard_map(
                _body,
                mesh=mesh,
                in_specs=(PartitionSpec("core"),) * (n_params + n_outs),
                out_specs=(PartitionSpec("core"),) * n_outs,
                check_rep=False,
            ),
            keep_unused=True,
        )
        self.sharding = NamedSharding(mesh, PartitionSpec("core"))
        self.zero_outs = zero_outs

    def stage(self, in_maps):
        """device_put concatenated per-core inputs + zero outputs once."""
        jax = self.jax
        concat = [
            np.concatenate([np.asarray(m[nm]) for m in in_maps], axis=0)
            for nm in self.in_names
        ]
        concat += [
            np.concatenate([z] * len(in_maps), axis=0) for z in self.zero_outs
        ]
        self.dev_args = [jax.device_put(a, self.sharding) for a in concat]
        jax.block_until_ready(self.dev_args)

    def run(self):
        out = self.sharded(*self.dev_args)
        self.jax.block_until_ready(out)
        return out


def _bf16(x):
    return np.ascontiguousarray(x, dtype=ml_dtypes.bfloat16)


def make_in_maps(
    query,
    key_,
    value,
    in_proj_weight,
    in_proj_bias,
    out_proj_bias,
    out_proj_weight=None,
    **kw,
):
    if out_proj_weight is None:
        out_proj_weight = kw["out_proj_weight"]
    query = np.asarray(query, np.float32)
    key_ = np.asarray(key_, np.float32)
    value = np.asarray(value, np.float32)
    W = np.asarray(in_proj_weight, np.float32)
    Wo = np.asarray(out_proj_weight, np.float32)
    bi = np.asarray(in_proj_bias, np.float32)
    assert not np.any(bi), "nonzero in_proj_bias unsupported by this kernel"

    scale = np.float32(1.0 / np.sqrt(D))
    wq, wk, wv = W[:E], W[E : 2 * E], W[2 * E :]

    # (E, BT) b-major transposed activations
    XTq = np.ascontiguousarray(query.transpose(2, 1, 0).reshape(E, TB))
    XTk = np.ascontiguousarray(key_.transpose(2, 1, 0).reshape(E, TB))
    XTv = np.ascontiguousarray(value.transpose(2, 1, 0).reshape(E, TB))

    mask = np.triu(np.ones((C, C), np.float32))  # U[s,t]=1 iff s<=t

    in_maps = []
    for core in range(N_CORES):
        hg, g = divmod(core, TBG)
        cols = np.r_[g * 256 : (g + 1) * 256, T + g * 256 : T + (g + 1) * 256]
        hsl = slice(hg * DHC, (hg + 1) * DHC)
        coefs = np.zeros((128, N_CORES), np.float32)
        for cid in range(N_CORES):
            if cid // TBG == hg and cid % TBG < g:
                coefs[:, cid] = 1.0
        in_maps.append(
            {
                "xtq": _bf16(XTq[:, cols]),
                "xtk": _bf16(XTk[:, cols]),
                "xtv": _bf16(XTv[:, cols]),
                "wqT": _bf16((wq[hsl, :] * scale).T),
                "wkT": _bf16(wk[hsl, :].T),
                "wvT": _bf16(wv[hsl, :].T),
                "woT": _bf16(Wo[:, hsl].T.copy()),
                "maskd": mask,
                "coefsd": coefs,
            }
        )
    return in_maps


def assemble_output(results, bo):
    out = np.zeros((T, B, E), np.float32)
    for core in range(N_CORES):
        hg, g = divmod(core, TBG)
        po = results[core]["pout"]  # (512, 1024) rows: b*256 + tl
        for b in range(B):
            out[g * 256 : (g + 1) * 256, b, :] += po[b * 256 : (b + 1) * 256, :]
    out += bo
    return out


def kernel(**inputs):
    in_maps = make_in_maps(**inputs)
    bo = np.asarray(inputs["out_proj_bias"], np.float32)
    nc = _build_nc()
    res = run_bass_kernel_spmd(nc, in_maps, list(range(N_CORES)))
    return assemble_output(res.results, bo)
